# revision 1
# baseline (speedup 1.0000x reference)
"""Trainium2 Bass kernel for nn_CGPBlock (attention block with 1x1-conv QKV).

Reference computation (per batch b):
    q = Wq @ pose + bq; k = Wk @ id + bk; v = Wv @ pose + bv     # [C, L]
    energy[i, j] = sum_c q[c, i] k[c, j]                          # [L, L]
    attn = softmax_j(energy)
    va[c, i] = sum_j v[c, j] attn[i, j]
    out = pose + gamma * va
Sharding: data-parallel over batch, B=8 batches -> 8 NeuronCores (SPMD).

Device algorithm (per core, matmuls bf16 with fp32 PSUM accumulate):
  - q' = (Wq@pose + bq) * S with S = 128/ln2, folded into the conv PSUM
    drain; the energy matmuls then produce E' = S*E directly.
  - exp work is split across two engines (ACT is otherwise the bottleneck
    at ~1.03us per [128,1024] tile):
      ACT tiles:  pt = exp(E' * (1/S))       (free scale on ACTIVATE)
      DVE tiles:  pt = bitcast_bf16(int16(E' + 16250))  -- Schraudolph:
                  bf16 bits of 2^(E'/128) ~ e^E; one tensor_scalar(add)
                  with int16 output (round-half-even on convert, measured).
                  |rel err| < 3.5%/elem, mean-centered via c=-6; softmax
                  normalization washes it out (end-to-end ~1e-3).
  - Z (softmax denominators, a cross-partition sum) via DVE oct pre-sums
    (7 bf16 adds per 8 tiles) + M=1 PE matmuls on the pre-sums.
  - Attention runs as ONE flat software pipeline over all 128 (chunk,
    j-tile) steps: the energy/exp stream runs SKEW tiles ahead of the
    va/Z stream, and chunk c+1's energy matmuls interleave with chunk
    c's va-flush in PE program order, so the PE never drains at chunk
    boundaries (the per-chunk version lost ~3.6us/boundary and let the
    HAM clock gate re-throttle).
  - out = pose' + gamma * va * (1/Z); pose' = pose + gamma*bv (v-bias
    folded into the residual; attention rows sum to 1). 1/Z broadcast
    across partitions via a DRAM round-trip (hidden under compute);
    last chunk uses a PE broadcast + 512-wide half-pipelined drain.
  - ~3.4us of garbage matmuls pre-warm the PE clock gate (HAM) during
    the input DMAs.
"""

import numpy as np
import ml_dtypes

import concourse.bacc as bacc
import concourse.tile as tile
from concourse import mybir
from concourse.bass_utils import run_bass_kernel_spmd

F32 = mybir.dt.float32
BF16 = mybir.dt.bfloat16
I16 = mybir.dt.int16
AF = mybir.ActivationFunctionType
ALU = mybir.AluOpType

B, C, L = 8, 128, 4096
CHUNK = 1024                # i-chunk width
NCH = L // CHUNK            # 4 chunks
NJT = L // 128              # 32 j-tiles per chunk
TOT = NCH * NJT             # 128 pipeline steps
JPC = CHUNK // 128          # j-tiles per chunk tile
QUAD = 4
SKEW = 8                    # energy/exp stream leads va/Z stream by SKEW

S = 184.66496280094524      # 128 / ln 2
INV_S = 1.0 / S
SCHC = 16250.0              # 16256 (bf16 exponent bias<<7) + c, c=-6 centers
                            # the Schraudolph frac-linearization error
# j-tiles (per chunk) computed on DVE instead of ACT; tuned so ACT ~ DVE
DVE_JT = frozenset({3, 10, 16, 22, 29})

_CACHE = {}


def _build():
    nc = bacc.Bacc("TRN2", target_bir_lowering=False, debug=False, num_devices=B)

    pose_d = nc.dram_tensor("pose", [C, L], F32, kind="ExternalInput").ap()
    posebf_d = nc.dram_tensor("posebf", [C, L], BF16, kind="ExternalInput").ap()
    idbf_d = nc.dram_tensor("idbf", [C, L], BF16, kind="ExternalInput").ap()
    wt_d = nc.dram_tensor("wt", [C, 3 * C], BF16, kind="ExternalInput").ap()
    bq_d = nc.dram_tensor("bq", [C, 1], F32, kind="ExternalInput").ap()
    bk_d = nc.dram_tensor("bk", [C, 1], F32, kind="ExternalInput").ap()
    bfin_d = nc.dram_tensor("bfin", [C, 1], F32, kind="ExternalInput").ap()
    gam_d = nc.dram_tensor("gam", [C, 1], F32, kind="ExternalInput").ap()
    out_d = nc.dram_tensor("out", [C, L], F32, kind="ExternalOutput").ap()

    with tile.TileContext(nc) as tc:
        with tc.tile_pool(name="res", bufs=1) as res:
            wt_sb = res.tile([C, 3 * C], BF16)
            nc.sync.dma_start(wt_sb, wt_d)
            bq_sb = res.tile([C, 1], F32)
            bk_sb = res.tile([C, 1], F32)
            bfin_sb = res.tile([C, 1], F32)
            nc.gpsimd.dma_start(bfin_sb, bfin_d)
            gam_sb = res.tile([C, 1], F32)
            nc.gpsimd.dma_start(gam_sb, gam_d)
            ones128_sb = res.tile([C, C], BF16)
            nc.vector.memset(ones128_sb, 1.0)

            def chunk_tiles(prefix, dtype):
                return [res.tile([C, CHUNK], dtype, name=f"{prefix}{i}")
                        for i in range(NCH)]

            pose_t = chunk_tiles("pose", F32)
            posebf_t = chunk_tiles("posebf", BF16)
            idbf_t = chunk_tiles("idbf", BF16)
            q_t = chunk_tiles("q", BF16)
            k_t = chunk_tiles("k", BF16)
            v_t = chunk_tiles("v", BF16)
            vt_t = chunk_tiles("vt", BF16)   # [j (partition), jt*128 + c]

            # input loads split across the two HWDGE queues (SP + ACT) so
            # posebf and idbf stream in parallel (~2us per 256KB chunk each)
            for ch in range(NCH):
                sl = slice(ch * CHUNK, (ch + 1) * CHUNK)
                nc.sync.dma_start(posebf_t[ch], posebf_d[:, sl])
                nc.scalar.dma_start(idbf_t[ch], idbf_d[:, sl])
                if ch == 1:
                    nc.sync.dma_start(bq_sb, bq_d)
                    nc.scalar.dma_start(bk_sb, bk_d)
            for ch in range(NCH):
                sl = slice(ch * CHUNK, (ch + 1) * CHUNK)
                nc.gpsimd.dma_start(pose_t[ch], pose_d[:, sl])

            # PE clock-gate pre-warm: ~3.4us of garbage matmuls (the HAM
            # SHORT window) so the convs run at 2.4 GHz; more filler is
            # interleaved between conv chunks below to bridge input-DMA
            # waits without letting the HAM re-throttle.
            warm_sb = res.tile([C, 512], BF16)
            nc.vector.memset(warm_sb, 0.0)

            wqT = wt_sb[:, 0:C]
            wkT = wt_sb[:, C:2 * C]
            wvT = wt_sb[:, 2 * C:3 * C]

            # ---- QKV convs (1x1 = channel-mixing matmuls) ----
            # Drains: v/k on ACT (idle until the first exp), q on DVE with
            # the bias-add and the Schraudolph pre-scale fused in.
            with tc.tile_pool(name="warm_ps", bufs=1, space="PSUM") as warm_ps, \
                 tc.tile_pool(name="conv_ps", bufs=3, space="PSUM") as conv_ps:
                wp = warm_ps.tile([C, 512], F32)

                def warm(n):
                    for _ in range(n):
                        nc.tensor.matmul(wp, lhsT=ones128_sb, rhs=warm_sb,
                                         start=True, stop=True)

                warm(10)
                for ch in range(NCH):
                    vp = conv_ps.tile([C, CHUNK], F32, tag="cv", name="vp")
                    kp = conv_ps.tile([C, CHUNK], F32, tag="cv", name="kp")
                    qp = conv_ps.tile([C, CHUNK], F32, tag="cv", name="qp")
                    for h in range(CHUNK // 512):
                        hs = slice(h * 512, (h + 1) * 512)
                        nc.tensor.matmul(vp[:, hs], lhsT=wvT,
                                         rhs=posebf_t[ch][:, hs],
                                         start=True, stop=True)
                        nc.tensor.matmul(kp[:, hs], lhsT=wkT,
                                         rhs=idbf_t[ch][:, hs],
                                         start=True, stop=True)
                        nc.tensor.matmul(qp[:, hs], lhsT=wqT,
                                         rhs=posebf_t[ch][:, hs],
                                         start=True, stop=True)
                    nc.scalar.copy(v_t[ch], vp)
                    nc.scalar.activation(k_t[ch], kp, AF.Identity, bias=bk_sb)
                    # q' = (q + bq) * S
                    nc.vector.tensor_scalar(q_t[ch], qp, bq_sb, S,
                                            op0=ALU.add, op1=ALU.mult)
                    # vT j-tiles via blockwise DMA xbar transpose
                    nc.sync.dma_start_transpose(
                        vt_t[ch].rearrange("p (t c) -> p t c", c=C),
                        v_t[ch])
                    if ch < NCH - 1:
                        warm(6)   # bridge the next chunk's DMA wait

            # pose' = pose + gamma*bv (per-partition const) — the residual
            for ch in range(NCH):
                nc.vector.tensor_scalar_add(pose_t[ch], pose_t[ch], bfin_sb)

            # ---- attention: one flat pipeline over all (chunk, j-tile) ----
            # Leading stream (paced by exp): energy matmuls + exp + Z
            # oct-presums + M=32 Z matmuls (lagged 2 steps so the PE never
            # waits on the DVE add chain) + the 1/Z reciprocal/broadcast
            # DMA as soon as each chunk's Z closes. Trailing stream (SKEW
            # behind): va matmuls; its flush hides every chunk's broadcast
            # round-trip, including the last one.
            with (
                tc.tile_pool(name="et_ps", bufs=2, space="PSUM") as et_ps,
                tc.tile_pool(name="va_ps", bufs=1, space="PSUM") as va_ps,
                tc.tile_pool(name="z_ps", bufs=1, space="PSUM") as z_ps,
                tc.tile_pool(name="pt_sb", bufs=SKEW + QUAD + 2) as pt_pool,
                tc.tile_pool(name="qs_sb", bufs=2) as qs_pool,
                tc.tile_pool(name="nrm", bufs=2) as nrm,
                tc.tile_pool(name="outb", bufs=2) as outb,
                tc.tile_pool(name="dramp", bufs=2, space="DRAM") as dramp,
            ):
                pts = {}
                va = None
                zs = {}          # chunk -> z psum tile
                rzbs = {}        # chunk -> broadcast 1/Z tile
                sab_prev = None
                lag_ptr = 0
                zq = []          # (due_g, fn) deferred Z matmuls / rz DMAs
                pending = []     # (due_lag, fn) deferred normalize finishes

                def z_matmul(ch, src, start, stop):
                    def fn(ch=ch, src=src, start=start, stop=stop):
                        z = zs[ch]
                        for h in range(CHUNK // 512):
                            hs = slice(h * 512, (h + 1) * 512)
                            nc.tensor.matmul(z[:, hs], lhsT=ones128_sb,
                                             rhs=src[:, hs],
                                             start=start, stop=stop)
                    return fn

                def z_close(ch):
                    # reciprocal + DRAM-round-trip broadcast of 1/Z; the
                    # trailing va stream hides the latency
                    def fn(ch=ch):
                        rz = nrm.tile([1, CHUNK], F32, tag="rz")
                        nc.vector.reciprocal_approx_fast(rz, zs[ch][0:1, :])
                        zd = dramp.tile([1, CHUNK], F32)
                        nc.sync.dma_start(zd, rz)
                        rzb = nrm.tile([C, CHUNK], F32, tag="rzb")
                        nc.sync.dma_start(rzb, zd.to_broadcast([C, CHUNK]))
                        rzbs[ch] = rzb
                    return fn

                def normalize_finish(ch, va_sb):
                    def fn(ch=ch, va_sb=va_sb):
                        isl = slice(ch * CHUNK, (ch + 1) * CHUNK)
                        t = nrm.tile([C, CHUNK], F32, tag="t")
                        nc.vector.tensor_mul(t, va_sb, rzbs[ch])
                        o = outb.tile([C, CHUNK], F32)
                        nc.vector.scalar_tensor_tensor(
                            o, in0=t, scalar=gam_sb, in1=pose_t[ch],
                            op0=ALU.mult, op1=ALU.add)
                        nc.sync.dma_start(out_d[:, isl], o)
                    return fn

                for g in range(TOT + 4):
                    while zq and zq[0][0] <= g:
                        zq.pop(0)[1]()
                    if g < TOT:
                        ch, jt = divmod(g, NJT)
                        ksl = slice((jt % JPC) * 128, (jt % JPC + 1) * 128)
                        if jt == 0:
                            zs[ch] = z_ps.tile([C, CHUNK], F32, name="z128")
                            sab_prev = None
                        et = et_ps.tile([C, CHUNK], F32)
                        for h in range(CHUNK // 512):
                            hs = slice(h * 512, (h + 1) * 512)
                            nc.tensor.matmul(
                                et[:, hs], lhsT=k_t[jt // JPC][:, ksl],
                                rhs=q_t[ch][:, hs],
                                start=True, stop=True)
                        pt = pt_pool.tile([C, CHUNK], BF16)
                        if jt in DVE_JT:
                            # Schraudolph: bf16 bits = round(E' + 16250)
                            nc.vector.tensor_scalar(
                                pt.bitcast(I16), et, SCHC, None, op0=ALU.add)
                        else:
                            nc.scalar.activation(pt, et, AF.Exp, scale=INV_S)
                        pts[g] = pt

                        # Z oct pre-sums (DVE) + deferred M=32 Z matmuls
                        if jt % QUAD == QUAD - 1:
                            qd = jt // QUAD
                            p0, p1, p2, p3 = (pts[g - 3], pts[g - 2],
                                              pts[g - 1], pts[g])
                            sa = qs_pool.tile([C, CHUNK], BF16, tag="sa")
                            nc.vector.tensor_add(sa, p0, p1)
                            sb_ = qs_pool.tile([C, CHUNK], BF16, tag="sb")
                            nc.vector.tensor_add(sb_, p2, p3)
                            sab = qs_pool.tile([C, CHUNK], BF16, tag="sab")
                            nc.vector.tensor_add(sab, sa, sb_)
                            if qd % 2 == 0:
                                sab_prev = sab
                            else:
                                s8 = qs_pool.tile([C, CHUNK], BF16, tag="s8")
                                nc.vector.tensor_add(s8, sab_prev, sab)
                                zq.append((g + 2, z_matmul(
                                    ch, s8, start=(qd == 1),
                                    stop=(qd == NJT // QUAD - 1))))
                                if qd == NJT // QUAD - 1:
                                    zq.append((g + 3, z_close(ch)))

                    # va stream runs SKEW behind; decay at the global tail
                    sk = SKEW if g < TOT - 6 else max(2, min(SKEW, TOT + 2 - g))
                    while lag_ptr <= min(g - sk, TOT - 1):
                        lag = lag_ptr
                        lag_ptr += 1
                        while pending and pending[0][0] <= lag:
                            pending.pop(0)[1]()
                        lch, jl = divmod(lag, NJT)
                        vsl = slice((jl % JPC) * 128, (jl % JPC + 1) * 128)
                        pt = pts.pop(lag)
                        if jl == 0:
                            va = va_ps.tile([C, CHUNK], F32)
                        for h in range(CHUNK // 512):
                            hs = slice(h * 512, (h + 1) * 512)
                            nc.tensor.matmul(
                                va[:, hs], lhsT=vt_t[jl // JPC][:, vsl],
                                rhs=pt[:, hs],
                                start=(jl == 0),
                                stop=(jl == NJT - 1))
                        if jl == NJT - 1:
                            va_sb = nrm.tile([C, CHUNK], F32, tag="va_sb")
                            nc.vector.tensor_copy(va_sb, va)
                            pending.append(
                                (lag + 6, normalize_finish(lch, va_sb)))
                for _, fn in zq:
                    fn()
                for _, fn in pending:
                    fn()

    nc.compile()
    return nc


def _get_nc():
    if "nc" not in _CACHE:
        _CACHE["nc"] = _build()
    return _CACHE["nc"]


def kernel(pose_f, id_f, Wq, bq, Wk, bk, Wv, bv, gamma, **run_kwargs):
    pose_f = np.asarray(pose_f, dtype=np.float32)
    id_f = np.asarray(id_f, dtype=np.float32)
    Wq = np.asarray(Wq, dtype=np.float32)
    Wk = np.asarray(Wk, dtype=np.float32)
    Wv = np.asarray(Wv, dtype=np.float32)
    bq = np.asarray(bq, dtype=np.float32)
    bk = np.asarray(bk, dtype=np.float32)
    bv = np.asarray(bv, dtype=np.float32)
    g = float(np.asarray(gamma, dtype=np.float32).reshape(-1)[0])

    bf = ml_dtypes.bfloat16
    wt = np.concatenate([Wq.T, Wk.T, Wv.T], axis=1).astype(bf)  # [C_in, 3C]
    posebf = pose_f.astype(bf)
    idbf = id_f.astype(bf)
    bq_c = np.ascontiguousarray(bq.reshape(C, 1))
    bk_c = np.ascontiguousarray(bk.reshape(C, 1))
    bfin = np.ascontiguousarray((g * bv).reshape(C, 1).astype(np.float32))
    gam = np.full((C, 1), g, dtype=np.float32)

    in_maps = []
    for b in range(B):
        in_maps.append({
            "pose": pose_f[b],
            "posebf": posebf[b],
            "idbf": idbf[b],
            "wt": wt,
            "bq": bq_c,
            "bk": bk_c,
            "bfin": bfin,
            "gam": gam,
        })

    nc = _get_nc()
    res = run_bass_kernel_spmd(nc, in_maps, core_ids=list(range(B)), **run_kwargs)
    out = np.stack([res.results[b]["out"] for b in range(B)], axis=0)
    if run_kwargs:
        _CACHE["last_result"] = res
    return out



# revision 18
# speedup vs baseline: 1.0290x; 1.0290x over previous
"""Trainium2 Bass kernel for nn_CGPBlock (attention block with 1x1-conv QKV).

Reference computation (per batch b):
    q = Wq @ pose + bq; k = Wk @ id + bk; v = Wv @ pose + bv     # [C, L]
    energy[i, j] = sum_c q[c, i] k[c, j]                          # [L, L]
    attn = softmax_j(energy)
    va[c, i] = sum_j v[c, j] attn[i, j]
    out = pose + gamma * va
Sharding: data-parallel over batch, B=8 batches -> 8 NeuronCores (SPMD).

Device algorithm (per core, matmuls bf16 with fp32 PSUM accumulate):
  - q' = (Wq@pose + bq) * S with S = 128/ln2, folded into the conv PSUM
    drain; the energy matmuls then produce E' = S*E directly.
  - exp work is split across two engines (ACT is otherwise the bottleneck
    at ~1.03us per [128,1024] tile):
      ACT tiles:  pt = exp(E' * (1/S))       (free scale on ACTIVATE)
      DVE tiles:  pt = bitcast_bf16(int16(E' + 16250))  -- Schraudolph:
                  bf16 bits of 2^(E'/128) ~ e^E; one tensor_scalar(add)
                  with int16 output (round-half-even on convert, measured).
                  |rel err| < 3.5%/elem, mean-centered via c=-6; softmax
                  normalization washes it out (end-to-end ~1e-3).
  - Z (softmax denominators, a cross-partition sum) via DVE oct pre-sums
    (7 bf16 adds per 8 tiles) + M=1 PE matmuls on the pre-sums.
  - Attention runs as ONE flat software pipeline over all 128 (chunk,
    j-tile) steps: the energy/exp stream runs SKEW tiles ahead of the
    va/Z stream, and chunk c+1's energy matmuls interleave with chunk
    c's va-flush in PE program order, so the PE never drains at chunk
    boundaries (the per-chunk version lost ~3.6us/boundary and let the
    HAM clock gate re-throttle).
  - out = pose' + gamma * va * (1/Z); pose' = pose + gamma*bv (v-bias
    folded into the residual; attention rows sum to 1). 1/Z broadcast
    across partitions via a DRAM round-trip (hidden under compute);
    last chunk uses a PE broadcast + 512-wide half-pipelined drain.
  - ~3.4us of garbage matmuls pre-warm the PE clock gate (HAM) during
    the input DMAs.
"""

import numpy as np
import ml_dtypes

import concourse.bacc as bacc
import concourse.tile as tile
from concourse import mybir
from concourse.bass_utils import run_bass_kernel_spmd

F32 = mybir.dt.float32
F32R = mybir.dt.float32r
BF16 = mybir.dt.bfloat16
I16 = mybir.dt.int16
AF = mybir.ActivationFunctionType
ALU = mybir.AluOpType

B, C, L = 8, 128, 4096
CHUNK = 1024                # i-chunk width
NCH = L // CHUNK            # 4 chunks
NJT = L // 128              # 32 j-tiles per chunk
TOT = NCH * NJT             # 128 pipeline steps
JPC = CHUNK // 128          # j-tiles per chunk tile
QUAD = 4
SKEW = 8                    # energy/exp stream leads va/Z stream by SKEW

S = 184.66496280094524      # 128 / ln 2
INV_S = 1.0 / S
SCHC = 16250.0              # 16256 (bf16 exponent bias<<7) + c, c=-6 centers
                            # the Schraudolph frac-linearization error
# j-tiles (per chunk) computed on DVE instead of ACT; tuned so ACT ~ DVE
DVE_JT = frozenset({3, 10, 16, 22, 29})

_CACHE = {}


def _build():
    nc = bacc.Bacc("TRN2", target_bir_lowering=False, debug=False, num_devices=B)

    pose_d = nc.dram_tensor("pose", [C, L], F32, kind="ExternalInput").ap()
    posebf_d = nc.dram_tensor("posebf", [C, L], BF16, kind="ExternalInput").ap()
    idbf_d = nc.dram_tensor("idbf", [C, L], BF16, kind="ExternalInput").ap()
    wt_d = nc.dram_tensor("wt", [C, 3 * C], BF16, kind="ExternalInput").ap()
    bq_d = nc.dram_tensor("bq", [C, 1], F32, kind="ExternalInput").ap()
    bk_d = nc.dram_tensor("bk", [C, 1], F32, kind="ExternalInput").ap()
    bfin_d = nc.dram_tensor("bfin", [C, 1], F32, kind="ExternalInput").ap()
    gam_d = nc.dram_tensor("gam", [C, 1], F32, kind="ExternalInput").ap()
    out_d = nc.dram_tensor("out", [C, L], F32, kind="ExternalOutput").ap()

    with tile.TileContext(nc) as tc:
        with tc.tile_pool(name="res", bufs=1) as res:
            wt_sb = res.tile([C, 3 * C], BF16)
            nc.sync.dma_start(wt_sb, wt_d)
            bq_sb = res.tile([C, 1], F32)
            bk_sb = res.tile([C, 1], F32)
            bfin_sb = res.tile([C, 1], F32)
            nc.gpsimd.dma_start(bfin_sb, bfin_d)
            gam_sb = res.tile([C, 1], F32)
            nc.gpsimd.dma_start(gam_sb, gam_d)
            ones128_sb = res.tile([C, C], BF16)
            nc.vector.memset(ones128_sb, 1.0)

            def chunk_tiles(prefix, dtype):
                return [res.tile([C, CHUNK], dtype, name=f"{prefix}{i}")
                        for i in range(NCH)]

            pose_t = chunk_tiles("pose", F32)
            posebf_t = chunk_tiles("posebf", BF16)
            idbf_t = chunk_tiles("idbf", BF16)
            q_t = chunk_tiles("q", BF16)
            k_t = chunk_tiles("k", BF16)
            v_t = chunk_tiles("v", BF16)
            vt_t = chunk_tiles("vt", BF16)   # [j (partition), jt*128 + c]

            # input loads split across the two HWDGE queues (SP + ACT) so
            # posebf and idbf stream in parallel (~2us per 256KB chunk each)
            for ch in range(NCH):
                sl = slice(ch * CHUNK, (ch + 1) * CHUNK)
                nc.sync.dma_start(posebf_t[ch], posebf_d[:, sl])
                nc.scalar.dma_start(idbf_t[ch], idbf_d[:, sl])
                if ch == 1:
                    nc.sync.dma_start(bq_sb, bq_d)
                    nc.scalar.dma_start(bk_sb, bk_d)
            for ch in range(NCH):
                sl = slice(ch * CHUNK, (ch + 1) * CHUNK)
                nc.gpsimd.dma_start(pose_t[ch], pose_d[:, sl])

            # PE clock-gate pre-warm: ~3.4us of garbage matmuls (the HAM
            # SHORT window) so the convs run at 2.4 GHz; more filler is
            # interleaved between conv chunks below to bridge input-DMA
            # waits without letting the HAM re-throttle.
            warm_sb = res.tile([C, 512], BF16)
            nc.vector.memset(warm_sb, 0.0)

            wqT = wt_sb[:, 0:C]
            wkT = wt_sb[:, C:2 * C]
            wvT = wt_sb[:, 2 * C:3 * C]

            # ---- QKV convs (1x1 = channel-mixing matmuls) ----
            # Drains: v/k on ACT (idle until the first exp), q on DVE with
            # the bias-add and the Schraudolph pre-scale fused in.
            with tc.tile_pool(name="warm_ps", bufs=1, space="PSUM") as warm_ps, \
                 tc.tile_pool(name="conv_ps", bufs=3, space="PSUM") as conv_ps:
                wp = warm_ps.tile([C, 512], F32)

                def warm(n):
                    for _ in range(n):
                        nc.tensor.matmul(wp, lhsT=ones128_sb, rhs=warm_sb,
                                         start=True, stop=True)

                warm(10)
                for ch in range(NCH):
                    vp = conv_ps.tile([C, CHUNK], F32, tag="cv", name="vp")
                    kp = conv_ps.tile([C, CHUNK], F32, tag="cv", name="kp")
                    qp = conv_ps.tile([C, CHUNK], F32, tag="cv", name="qp")
                    for h in range(CHUNK // 512):
                        hs = slice(h * 512, (h + 1) * 512)
                        nc.tensor.matmul(vp[:, hs], lhsT=wvT,
                                         rhs=posebf_t[ch][:, hs],
                                         start=True, stop=True)
                        nc.tensor.matmul(kp[:, hs], lhsT=wkT,
                                         rhs=idbf_t[ch][:, hs],
                                         start=True, stop=True)
                        nc.tensor.matmul(qp[:, hs], lhsT=wqT,
                                         rhs=posebf_t[ch][:, hs],
                                         start=True, stop=True)
                    nc.scalar.copy(v_t[ch], vp)
                    nc.scalar.activation(k_t[ch], kp, AF.Identity, bias=bk_sb)
                    # q' = (q + bq) * S
                    nc.vector.tensor_scalar(q_t[ch], qp, bq_sb, S,
                                            op0=ALU.add, op1=ALU.mult)
                    # vT j-tiles via blockwise DMA xbar transpose
                    nc.sync.dma_start_transpose(
                        vt_t[ch].rearrange("p (t c) -> p t c", c=C),
                        v_t[ch])
                    if ch < NCH - 1:
                        warm(6)   # bridge the next chunk's DMA wait

            # pose' = pose + gamma*bv (per-partition const) — the residual
            for ch in range(NCH):
                nc.vector.tensor_scalar_add(pose_t[ch], pose_t[ch], bfin_sb)

            # ---- attention: one flat pipeline over all (chunk, j-tile) ----
            # Leading stream (paced by exp): energy matmuls + exp + Z
            # oct-presums + M=32 Z matmuls (lagged 2 steps so the PE never
            # waits on the DVE add chain) + the 1/Z reciprocal/broadcast
            # DMA as soon as each chunk's Z closes. Trailing stream (SKEW
            # behind): va matmuls; its flush hides every chunk's broadcast
            # round-trip, including the last one.
            with (
                tc.tile_pool(name="et_ps", bufs=2, space="PSUM") as et_ps,
                tc.tile_pool(name="va_ps", bufs=1, space="PSUM") as va_ps,
                tc.tile_pool(name="z_ps", bufs=1, space="PSUM") as z_ps,
                tc.tile_pool(name="pt_sb", bufs=SKEW + QUAD + 2) as pt_pool,
                tc.tile_pool(name="qs_sb", bufs=2) as qs_pool,
                tc.tile_pool(name="nrm", bufs=2) as nrm,
                tc.tile_pool(name="outb", bufs=2) as outb,
                tc.tile_pool(name="dramp", bufs=2, space="DRAM") as dramp,
            ):
                pts = {}
                va = None
                zs = {}          # chunk -> z psum tile
                rzbs = {}        # chunk -> broadcast 1/Z tile
                sab_prev = None
                lag_ptr = 0
                zq = []          # (due_g, fn) deferred Z matmuls / rz DMAs
                pending = []     # (due_lag, fn) deferred normalize finishes
                LC = NCH - 1     # last chunk: PE-based Z close + broadcast
                last = {}        # va psum + gamma/Z psum tiles for last chunk

                def z_matmul(ch, src, start, stop):
                    def fn(ch=ch, src=src, start=start, stop=stop):
                        z = zs[ch]
                        for h in range(CHUNK // 512):
                            hs = slice(h * 512, (h + 1) * 512)
                            nc.tensor.matmul(z[:, hs], lhsT=ones128_sb,
                                             rhs=src[:, hs],
                                             start=start, stop=stop)
                    return fn

                def last_close():
                    # last chunk, no DRAM round trip: ACT drains Z row to
                    # bf16, PE outer-product-broadcasts it across the 128
                    # partitions, DVE reciprocal drains 1/Z to SBUF.
                    zrow = nrm.tile([1, CHUNK], BF16, tag="zrow")
                    nc.scalar.copy(zrow, zs[LC][0:1, :])
                    zb = et_ps.tile([C, CHUNK], F32, name="et")
                    rzs = nrm.tile([C, CHUNK], F32, tag="rzs")
                    last["rzs"] = rzs
                    for h in range(CHUNK // 512):
                        hs = slice(h * 512, (h + 1) * 512)
                        nc.tensor.matmul(zb[:, hs],
                                         lhsT=ones128_sb[0:1, :],
                                         rhs=zrow[:, hs],
                                         start=True, stop=True)
                        nc.vector.reciprocal_approx_fast(rzs[:, hs],
                                                         zb[:, hs])

                def norm_last():
                    # normalize straight from PSUM in halves so the out DMA
                    # overlaps the second half's DVE work
                    for h in range(CHUNK // 512):
                        hs = slice(h * 512, (h + 1) * 512)
                        osl = slice(LC * CHUNK + h * 512,
                                    LC * CHUNK + (h + 1) * 512)
                        t = nrm.tile([C, 512], F32, tag="tl")
                        nc.vector.tensor_mul(t, last["va"][:, hs],
                                             last["rzs"][:, hs])
                        o = outb.tile([C, 512], F32, tag="ol")
                        nc.vector.scalar_tensor_tensor(
                            o, in0=t, scalar=gam_sb, in1=pose_t[LC][:, hs],
                            op0=ALU.mult, op1=ALU.add)
                        nc.sync.dma_start(out_d[:, osl], o)

                def z_close(ch):
                    # reciprocal + DRAM-round-trip broadcast of 1/Z; the
                    # trailing va stream hides the latency
                    def fn(ch=ch):
                        rz = nrm.tile([1, CHUNK], F32, tag="rz")
                        nc.vector.reciprocal_approx_fast(rz, zs[ch][0:1, :])
                        zd = dramp.tile([1, CHUNK], F32)
                        nc.sync.dma_start(zd, rz)
                        rzb = nrm.tile([C, CHUNK], F32, tag="rzb")
                        nc.sync.dma_start(rzb, zd.to_broadcast([C, CHUNK]))
                        rzbs[ch] = rzb
                    return fn

                def normalize_finish(ch, va_sb):
                    def fn(ch=ch, va_sb=va_sb):
                        isl = slice(ch * CHUNK, (ch + 1) * CHUNK)
                        t = nrm.tile([C, CHUNK], F32, tag="t")
                        nc.vector.tensor_mul(t, va_sb, rzbs[ch])
                        o = outb.tile([C, CHUNK], F32)
                        nc.vector.scalar_tensor_tensor(
                            o, in0=t, scalar=gam_sb, in1=pose_t[ch],
                            op0=ALU.mult, op1=ALU.add)
                        nc.sync.dma_start(out_d[:, isl], o)
                    return fn

                for g in range(TOT + 4):
                    while zq and zq[0][0] <= g:
                        zq.pop(0)[1]()
                    if g < TOT:
                        ch, jt = divmod(g, NJT)
                        ksl = slice((jt % JPC) * 128, (jt % JPC + 1) * 128)
                        if jt == 0:
                            zs[ch] = z_ps.tile([C, CHUNK], F32, name="z128")
                            sab_prev = None
                        et = et_ps.tile([C, CHUNK], F32)
                        for h in range(CHUNK // 512):
                            hs = slice(h * 512, (h + 1) * 512)
                            nc.tensor.matmul(
                                et[:, hs], lhsT=k_t[jt // JPC][:, ksl],
                                rhs=q_t[ch][:, hs],
                                start=True, stop=True)
                        pt = pt_pool.tile([C, CHUNK], BF16)
                        if jt in DVE_JT:
                            # Schraudolph: bf16 bits = round(E' + 16250)
                            nc.vector.tensor_scalar(
                                pt.bitcast(I16), et, SCHC, None, op0=ALU.add)
                        else:
                            nc.scalar.activation(pt, et, AF.Exp, scale=INV_S)
                        pts[g] = pt

                        # Z oct pre-sums (DVE) + deferred M=32 Z matmuls.
                        # Last chunk: quad 6 closes via its own sab matmul
                        # and quad 7 via direct per-tile matmuls, so no DVE
                        # presum work remains after the final exp and the
                        # reciprocal can fire immediately.
                        if ch == LC and jt >= NJT - QUAD:
                            zq.append((g + 2, z_matmul(
                                ch, pt, start=False, stop=(jt == NJT - 1))))
                            if jt == NJT - 1:
                                zq.append((g + 3, last_close))
                        elif jt % QUAD == QUAD - 1:
                            qd = jt // QUAD
                            p0, p1, p2, p3 = (pts[g - 3], pts[g - 2],
                                              pts[g - 1], pts[g])
                            sa = qs_pool.tile([C, CHUNK], BF16, tag="sa")
                            nc.vector.tensor_add(sa, p0, p1)
                            sb_ = qs_pool.tile([C, CHUNK], BF16, tag="sb")
                            nc.vector.tensor_add(sb_, p2, p3)
                            sab = qs_pool.tile([C, CHUNK], BF16, tag="sab")
                            nc.vector.tensor_add(sab, sa, sb_)
                            if ch == LC and qd == NJT // QUAD - 2:
                                zq.append((g + 2, z_matmul(
                                    ch, sab, start=False, stop=False)))
                            elif qd % 2 == 0:
                                sab_prev = sab
                            else:
                                s8 = qs_pool.tile([C, CHUNK], BF16, tag="s8")
                                nc.vector.tensor_add(s8, sab_prev, sab)
                                zq.append((g + 2, z_matmul(
                                    ch, s8, start=(qd == 1),
                                    stop=(ch != LC
                                          and qd == NJT // QUAD - 1))))
                                if qd == NJT // QUAD - 1 and ch != LC:
                                    zq.append((g + 3, z_close(ch)))

                    # va stream runs SKEW behind; decay at the global tail
                    sk = SKEW if g < TOT - 6 else max(2, min(SKEW, TOT + 2 - g))
                    while lag_ptr <= min(g - sk, TOT - 1):
                        lag = lag_ptr
                        lag_ptr += 1
                        while pending and pending[0][0] <= lag:
                            pending.pop(0)[1]()
                        lch, jl = divmod(lag, NJT)
                        vsl = slice((jl % JPC) * 128, (jl % JPC + 1) * 128)
                        pt = pts.pop(lag)
                        if jl == 0:
                            va = va_ps.tile([C, CHUNK], F32)
                        for h in range(CHUNK // 512):
                            hs = slice(h * 512, (h + 1) * 512)
                            nc.tensor.matmul(
                                va[:, hs], lhsT=vt_t[jl // JPC][:, vsl],
                                rhs=pt[:, hs],
                                start=(jl == 0),
                                stop=(jl == NJT - 1))
                        if jl == NJT - 1:
                            if lch == LC:
                                last["va"] = va
                                zq.append((g + 2, norm_last))
                            else:
                                va_sb = nrm.tile([C, CHUNK], F32,
                                                 tag="va_sb")
                                nc.vector.tensor_copy(va_sb, va)
                                pending.append(
                                    (lag + 6, normalize_finish(lch, va_sb)))
                for _, fn in zq:
                    fn()
                for _, fn in pending:
                    fn()

    nc.compile()
    return nc


def _get_nc():
    if "nc" not in _CACHE:
        _CACHE["nc"] = _build()
    return _CACHE["nc"]


def kernel(pose_f, id_f, Wq, bq, Wk, bk, Wv, bv, gamma, **run_kwargs):
    pose_f = np.asarray(pose_f, dtype=np.float32)
    id_f = np.asarray(id_f, dtype=np.float32)
    Wq = np.asarray(Wq, dtype=np.float32)
    Wk = np.asarray(Wk, dtype=np.float32)
    Wv = np.asarray(Wv, dtype=np.float32)
    bq = np.asarray(bq, dtype=np.float32)
    bk = np.asarray(bk, dtype=np.float32)
    bv = np.asarray(bv, dtype=np.float32)
    g = float(np.asarray(gamma, dtype=np.float32).reshape(-1)[0])

    bf = ml_dtypes.bfloat16
    wt = np.concatenate([Wq.T, Wk.T, Wv.T], axis=1).astype(bf)  # [C_in, 3C]
    posebf = pose_f.astype(bf)
    idbf = id_f.astype(bf)
    bq_c = np.ascontiguousarray(bq.reshape(C, 1))
    bk_c = np.ascontiguousarray(bk.reshape(C, 1))
    bfin = np.ascontiguousarray((g * bv).reshape(C, 1).astype(np.float32))
    gam = np.full((C, 1), g, dtype=np.float32)

    in_maps = []
    for b in range(B):
        in_maps.append({
            "pose": pose_f[b],
            "posebf": posebf[b],
            "idbf": idbf[b],
            "wt": wt,
            "bq": bq_c,
            "bk": bk_c,
            "bfin": bfin,
            "gam": gam,
        })

    nc = _get_nc()
    res = run_bass_kernel_spmd(nc, in_maps, core_ids=list(range(B)), **run_kwargs)
    out = np.stack([res.results[b]["out"] for b in range(B)], axis=0)
    if run_kwargs:
        _CACHE["last_result"] = res
    return out

